# revision 22
# baseline (speedup 1.0000x reference)
"""Trainium2 Bass kernel for nn_AnatomicalContrastiveLoss.

Distribution: V (voxel) dim sharded 8 ways; every core holds all B,C,F.
Pipeline per core (engine assignment in brackets):
  - proba -> weights -> local top-32/b [sync DMA + DVE] -> AllGather#1
    fired ~15us in [gpsimd]; merge + index recovery run mid-stream.
  - emb streamed as 8 x [128, 8192] f32 chunks (b-pairs) [sync HWDGE],
    PE-transposed in [128,128] blocks (f32, 2cyc/row), PSUM->SBUF copy
    with fp8 cast split DVE/scalar, fp8 class-sum matmuls accumulate in
    PSUM, fp8 ebT evicted to DRAM in a (v%128)-major layout so each
    eviction is 4KB-contiguous per partition [sync].
  - tail: batched indirect he/sel gathers [gpsimd], AllGather#2 of
    class_sum/count [gpsimd], EMA -> nar -> loss epilogue.
Host combines per-core loss partials (disjoint support), slices [:, :100].
"""

import numpy as np

import concourse.bass as bass
import concourse.tile as tile
from concourse import bacc, mybir
from concourse.bass import IndirectOffsetOnAxis
from concourse.masks import make_identity

B, C, V, F, K = 4, 4, 262144, 64, 100
NCORES = 8
VL = V // NCORES          # 32768
R1 = 3                    # L1 rounds: top-24 per 1024-voxel partition
R2 = 4                    # L2 rounds: top-32 per core
R3 = 13                   # L3 rounds: top-104 global
KE = 8 * R3               # 104 extracted globally
NL = 8 * R2               # 32 local candidates per b
EMA_THETA = 0.9
TAU = 0.1
SENT = -1.0               # removal sentinel (< any weight; weights in (0,1))
NOTFOUND_CLAMP = 2.0e7    # > B*VL, makes unowned gather indices OOB-skipped
CH = 8192                 # emb chunk columns (voxels per chunk)
NCHUNK = 2 * (VL // CH)   # 2 b-pairs x 4 h-slices = 8
TDIV = VL // 128          # 256 rows per vp in sigma layout

f32 = mybir.dt.float32
bf16 = mybir.dt.bfloat16
fp8 = mybir.dt.float8e4
i32 = mybir.dt.int32
u32 = mybir.dt.uint32
Alu = mybir.AluOpType
Act = mybir.ActivationFunctionType
Axis = mybir.AxisListType


def build_graph(debug_taps=False):
    nc = bacc.Bacc("TRN2", target_bir_lowering=False, debug=False,
                   num_devices=NCORES)

    proba_d = nc.dram_tensor("proba", [B, C, VL], f32, kind="ExternalInput")
    y_d = nc.dram_tensor("y", [B, C, VL], i32, kind="ExternalInput")
    emb_d = nc.dram_tensor("embeddings", [B, F, VL], f32, kind="ExternalInput")
    avg_d = nc.dram_tensor("avg_repr", [1, C, F], f32, kind="ExternalInput")
    out_d = nc.dram_tensor("out", [B, KE], f32, kind="ExternalOutput")

    taps = {}

    def tap(name, shape, dtype=f32):
        if not debug_taps:
            return None
        t = nc.dram_tensor(name, shape, dtype, kind="ExternalOutput")
        taps[name] = t
        return t.ap()

    with tile.TileContext(nc) as tc:
        _build(tc, proba_d.ap(), y_d.ap(), emb_d.ap(), avg_d.ap(), out_d.ap(),
               tap)

    nc.compile()
    return nc


def _build(tc, proba_d, y_d, emb_d, avg_d, out_d, tap=lambda *a, **k: None):
    nc = tc.nc

    def emit_tap(name, src_ap, shape, dtype=f32):
        t = tap(name, shape, dtype)
        if t is not None:
            nc.scalar.dma_start(t, src_ap)

    import contextlib
    ctx = contextlib.ExitStack()
    with ctx:
        sb = ctx.enter_context(tc.tile_pool(name="sb", bufs=1))
        sb2 = ctx.enter_context(tc.tile_pool(name="sb2", bufs=2))
        sbch = ctx.enter_context(tc.tile_pool(name="sbch", bufs=2))
        sbt = ctx.enter_context(tc.tile_pool(name="sbt", bufs=2))
        ps = ctx.enter_context(tc.tile_pool(name="ps", bufs=2, space="PSUM"))
        dram = ctx.enter_context(tc.tile_pool(name="dram", bufs=1,
                                              space="DRAM"))

        # ============ DRAM scratch ============
        embT_dram = dram.tile([B, 128, TDIV, F], fp8)   # sigma layout
        s_dram = dram.tile([B * VL, 1], f32)
        b1_in = dram.tile([B, NL], f32)
        b1_out = dram.tile([NCORES, B, NL], f32, addr_space="Shared")
        b2_in = dram.tile([C, F + 1], f32)
        b2_out = dram.tile([NCORES, C, F + 1], f32, addr_space="Shared")

        # ============ input loads (issue order sets priority) ============
        proba_sb = sb.tile([128, C, 1024], f32)
        for c in range(C):
            nc.sync.dma_start(proba_sb[:, c, :], proba_d[:, c, :])
        # y as [(b c s), 4096]: p = 32b + 8c + s, v = s*4096 + col
        # SWDGE cast-load i32 -> bf16 gives the mask directly (y in {0,1})
        mask8 = sb.tile([128, 4096], bf16)
        nc.gpsimd.dma_start(
            mask8[:], y_d.rearrange("b c (s v) -> (b c s) v", s=8))

        chunks = {}

        def load_chunk(ci):
            p, h = ci % 2, ci // 2
            t = sbch.tile([128, CH], f32, tag="ch", name=f"ch{ci}")
            nc.sync.dma_start(
                t[:],
                emb_d[2 * p:2 * p + 2, :,
                      h * CH:(h + 1) * CH].rearrange("b f v -> (b f) v"))
            chunks[ci] = t

        for ci in range(2):
            load_chunk(ci)

        # ============ constants (gpsimd + DVE, overlap loads) ============
        ident = sb.tile([128, 128], f32)
        make_identity(nc, ident[:])
        ident_bf = sb.tile([128, 128], bf16)
        nc.vector.tensor_copy(ident_bf[:], ident[:])

        iota_p = sb.tile([128, 1], i32)
        nc.gpsimd.iota(iota_p[:], pattern=[[1, 1]], base=0,
                       channel_multiplier=1)
        iota_pf = sb.tile([128, 1], f32)
        nc.vector.tensor_copy(iota_pf[:], iota_p[:])

        def floor_f(x_ap, shape, tg):
            # in-place floor: f32->i32 cast ROUNDS to nearest, so correct
            # by subtracting 1 where the rounded value exceeds the input
            xi = sb2.tile(shape, i32, tag=tg + "i")
            nc.vector.tensor_copy(xi[:], x_ap)
            xr = sb2.tile(shape, f32, tag=tg + "r")
            nc.vector.tensor_copy(xr[:], xi[:])
            corr = sb2.tile(shape, f32, tag=tg + "c")
            nc.vector.tensor_tensor(corr[:], xr[:], x_ap, op=Alu.is_gt)
            nc.vector.tensor_tensor(x_ap, xr[:], corr[:], op=Alu.subtract)

        # b(p) = p//32, s8(p) = p%8
        b_of_p = sb.tile([128, 1], f32)
        nc.vector.tensor_scalar(b_of_p[:], iota_pf[:], 1.0 / 32.0, None,
                                op0=Alu.mult)
        floor_f(b_of_p[:], [128, 1], "fb")
        r32_of_p = sb.tile([128, 1], f32)
        nc.vector.scalar_tensor_tensor(r32_of_p[:], in0=b_of_p[:],
                                       scalar=-32.0, in1=iota_pf[:],
                                       op0=Alu.mult, op1=Alu.add)
        q8_of_p = sb.tile([128, 1], f32)
        nc.vector.tensor_scalar(q8_of_p[:], iota_pf[:], 1.0 / 8.0, None,
                                op0=Alu.mult)
        floor_f(q8_of_p[:], [128, 1], "fq")
        s8_of_p = sb.tile([128, 1], f32)
        nc.vector.scalar_tensor_tensor(s8_of_p[:], in0=q8_of_p[:],
                                       scalar=-8.0, in1=iota_pf[:],
                                       op0=Alu.mult, op1=Alu.add)

        # L4[b, j] = 1 iff j//32 == b
        col128_i = sb.tile([B, 128], i32)
        nc.gpsimd.iota(col128_i[:], pattern=[[1, 128]], base=0,
                       channel_multiplier=0)
        col128 = sb.tile([B, 128], f32)
        nc.vector.tensor_copy(col128[:], col128_i[:])
        bcol4 = sb.tile([B, 1], i32)
        nc.gpsimd.iota(bcol4[:], pattern=[[1, 1]], base=0,
                       channel_multiplier=1)
        bcol4f = sb.tile([B, 1], f32)
        nc.vector.tensor_copy(bcol4f[:], bcol4[:])
        j32 = sb.tile([B, 128], f32)
        nc.vector.tensor_scalar(j32[:], col128[:], 1.0 / 32.0, None,
                                op0=Alu.mult)
        floor_f(j32[:], [B, 128], "fl4")
        L4 = sb.tile([B, 128], f32)
        nc.vector.tensor_tensor(L4[:], j32[:],
                                bcol4f[:].to_broadcast([B, 128]),
                                op=Alu.is_equal)

        # S4[p, j] = 1 iff j == 8*(p//32) + p%8
        col32_i = sb.tile([128, 32], i32)
        nc.gpsimd.iota(col32_i[:], pattern=[[1, 32]], base=0,
                       channel_multiplier=0)
        col32 = sb.tile([128, 32], f32)
        nc.vector.tensor_copy(col32[:], col32_i[:])
        jtgt = sb.tile([128, 1], f32)
        nc.vector.scalar_tensor_tensor(jtgt[:], in0=b_of_p[:], scalar=8.0,
                                       in1=s8_of_p[:], op0=Alu.mult,
                                       op1=Alu.add)
        S_CAT = sb.tile([128, 64], bf16)
        nc.vector.tensor_tensor(S_CAT[:, 0:32], col32[:],
                                jtgt[:].to_broadcast([128, 32]),
                                op=Alu.is_equal)

        # G2[p, c] = 1 iff c == (p%32)//8
        colC_i = sb.tile([128, C], i32)
        nc.gpsimd.iota(colC_i[:], pattern=[[1, C]], base=0,
                       channel_multiplier=0)
        colC = sb.tile([128, C], f32)
        nc.vector.tensor_copy(colC[:], colC_i[:])
        c_of_p = sb.tile([128, 1], f32)
        nc.vector.tensor_scalar(c_of_p[:], r32_of_p[:], 1.0 / 8.0, None,
                                op0=Alu.mult)
        floor_f(c_of_p[:], [128, 1], "fg2")
        G2 = sb.tile([128, C], f32)
        nc.vector.tensor_tensor(G2[:], colC[:],
                                c_of_p[:].to_broadcast([128, C]),
                                op=Alu.is_equal)
        # S0 = S4 masked to c==0 rows: S_CAT[:, 32:64]
        c0ind = sb.tile([128, 1], f32)
        nc.vector.tensor_scalar(c0ind[:], c_of_p[:], 0.0, None,
                                op0=Alu.is_equal)
        nc.vector.tensor_tensor(S_CAT[:, 32:64], S_CAT[:, 0:32],
                                c0ind[:].to_broadcast([128, 32]),
                                op=Alu.mult)

        ones8 = sb.tile([NCORES, 1], f32)
        nc.gpsimd.memset(ones8[:], 1.0)
        ones104 = sb.tile([1, KE], f32)
        nc.gpsimd.memset(ones104[:], 1.0)
        p1024 = sb.tile([128, 1], f32)
        nc.vector.tensor_scalar(p1024[:], iota_pf[:], 1024.0, None,
                                op0=Alu.mult)
        bsub = sb.tile([128, 1], f32)
        nc.vector.tensor_scalar(bsub[:], b_of_p[:], float(VL), None,
                                op0=Alu.mult)
        # bcol[k, b] = b*VL
        bcol_i = sb.tile([KE, B], i32)
        nc.gpsimd.iota(bcol_i[:], pattern=[[1, B]], base=0,
                       channel_multiplier=0)
        bcol = sb.tile([KE, B], f32)
        nc.vector.tensor_copy(bcol[:], bcol_i[:])
        nc.vector.tensor_scalar_mul(bcol[:], bcol[:], float(VL))

        # ============ candidate path (DVE) ============
        # Worig/Wx live inside proba_sb (c=0 / c=1 slots) to save SBUF
        Worig = proba_sb[:, 0, :]
        nc.vector.tensor_tensor(Worig, proba_sb[:, 0, :],
                                proba_sb[:, 1, :], op=Alu.mult)
        nc.vector.tensor_tensor(Worig, Worig, proba_sb[:, 2, :],
                                op=Alu.mult)
        nc.vector.tensor_tensor(Worig, Worig, proba_sb[:, 3, :],
                                op=Alu.mult)
        Wx = proba_sb[:, 1, :]
        nc.vector.tensor_copy(Wx, proba_sb[:, 0, :])
        V1 = sb.tile([128, 8 * R1], f32)
        for r in range(R1):
            sl = V1[:, r * 8:(r + 1) * 8]
            nc.vector.max(out=sl, in_=Wx)
            nc.vector.match_replace(out=Wx, in_to_replace=sl,
                                    in_values=Wx, imm_value=SENT)
        cand = sb.tile([B, 32 * 8 * R1], f32)
        nc.scalar.dma_start(cand[:], V1[:])
        L2V = sb.tile([B, NL], f32)
        for r in range(R2):
            sl = L2V[:, r * 8:(r + 1) * 8]
            nc.vector.max(out=sl, in_=cand[:])
            nc.vector.match_replace(out=cand[:], in_to_replace=sl,
                                    in_values=cand[:], imm_value=SENT)
        emit_tap("d_L2V", L2V[:], [B, NL])
        nc.gpsimd.dma_start(b1_in[:], L2V[:])
        # AllGather #1 — fired early; blocks gpsimd until done (~45us);
        # nothing else is queued on gpsimd until gcand/tail.
        nc.gpsimd.collective_compute(
            "AllGather", Alu.bypass,
            replica_groups=[list(range(NCORES))],
            ins=[b1_in[:].opt()], outs=[b1_out[:].opt()])

        # ============ mask path (fp8) ============
        maskT = sb.tile([128, 32, 128], fp8)
        for g in range(8):
            mt_ps = ps.tile([128, 512], bf16, tag="pstr", bufs=3,
                            name=f"mt{g}")
            for q in range(4):
                s = g * 4 + q
                nc.tensor.transpose(mt_ps[:, q * 128:(q + 1) * 128],
                                    mask8[:, s * 128:(s + 1) * 128],
                                    ident_bf[:])
            nc.vector.tensor_copy(maskT[:, g * 4:(g + 1) * 4, :], mt_ps[:])
        emit_tap("d_maskT", maskT[:, 0, :], [128, 128], fp8)

        cnt_part = sb.tile([128, 1], f32)
        nc.vector.reduce_sum(cnt_part[:], mask8[:], axis=Axis.X)
        cnt_ps = ps.tile([C, 1], f32, tag="psa")
        nc.tensor.matmul(cnt_ps[:], lhsT=G2[:], rhs=cnt_part[:],
                         start=True, stop=True)
        cnt_sb = sb.tile([C, 1], f32)
        nc.vector.tensor_copy(cnt_sb[:], cnt_ps[:])
        emit_tap("d_cnt", cnt_sb[:], [C, 1])

        # sel field: s[b,v] = 1 if (no class active) or (class0 active)
        # one matmul computes [msum | m0] stacked on 64 psum partitions
        sval = sb.tile([32, 4096], f32)
        for q in range(8):
            ms_ps = ps.tile([64, 512], f32, tag="psb", bufs=1,
                            name=f"ms{q}")
            nc.tensor.matmul(ms_ps[:], lhsT=S_CAT[:],
                             rhs=mask8[:, 512 * q:512 * (q + 1)],
                             start=True, stop=True)
            m0sb = sb2.tile([32, 512], f32, tag="m0sb")
            nc.vector.tensor_copy(m0sb[:], ms_ps[32:64, :])
            nc.vector.scalar_tensor_tensor(
                sval[:, 512 * q:512 * (q + 1)], in0=ms_ps[0:32, :],
                scalar=0.0, in1=m0sb[:], op0=Alu.is_equal,
                op1=Alu.max)
        nc.scalar.dma_start(
            s_dram[:].rearrange("(p v) a -> p (v a)", p=32), sval[:])

        # ============ emb streaming ============
        csA = ps.tile([8, 128], f32, tag="pscsA", bufs=1)
        csB = ps.tile([8, 128], f32, tag="pscsB", bufs=1)
        first_mm = {0: True, 1: True}
        n_mm = {0: 0, 1: 0}
        copy_flip = [0]

        def stream_chunk(ci):
            p, h = ci % 2, ci // 2
            t = chunks.pop(ci)
            ebT = sbt.tile([128, 64, 128], fp8, tag="ebT", name=f"ebT{ci}")
            cs_psum = csA if p == 0 else csB
            for g in range(16):
                tp_ps = ps.tile([128, 512], f32, tag="pstr", bufs=3,
                                name=f"tp{ci}_{g}")
                for q in range(4):
                    vb = g * 4 + q
                    nc.tensor.transpose(tp_ps[:, q * 128:(q + 1) * 128],
                                        t[:, vb * 128:(vb + 1) * 128],
                                        ident[:])
                dst = ebT[:, g * 4:(g + 1) * 4, :]
                if copy_flip[0] % 2 == 0:
                    nc.vector.tensor_copy(dst, tp_ps[:])
                else:
                    nc.scalar.activation(dst, tp_ps[:], Act.Copy)
                copy_flip[0] += 1
                for q in range(4):
                    vb = g * 4 + q
                    v0 = h * CH + vb * 128
                    s8, s = v0 // 4096, (v0 % 4096) // 128
                    c0 = s8 + 64 * p
                    lhsT = maskT[:, s, c0:c0 + 57:8]  # 8 cols: pair's (b,c)
                    n_mm[p] += 1
                    nc.tensor.matmul(
                        cs_psum[:], lhsT=lhsT,
                        rhs=ebT[:, vb, :],
                        start=first_mm[p],
                        stop=(n_mm[p] == (VL // 128)),
                        skip_group_check=True)
                    first_mm[p] = False
            # eviction: sigma layout, 4KB contiguous per partition
            for half in range(2):
                b = 2 * p + half
                nc.sync.dma_start(
                    embT_dram[b, :, h * 64:(h + 1) * 64, :],
                    ebT[:, :, half * F:(half + 1) * F])
            if ci + 2 < NCHUNK:
                load_chunk(ci + 2)

        for ci in range(4):
            stream_chunk(ci)

        # ===== merge + index recovery (mid-stream; gcand via gpsimd) =====
        gcand = sb.tile([B, NCORES * NL], f32)
        nc.gpsimd.dma_start(
            gcand[:].rearrange("b (ci r) -> b ci r", ci=NCORES),
            b1_out[:].rearrange("ci b r -> b ci r"))
        emit_tap("d_gcand", gcand[:], [B, NCORES * NL])
        G = sb.tile([B, KE], f32)
        for r in range(R3):
            sl = G[:, r * 8:(r + 1) * 8]
            nc.vector.max(out=sl, in_=gcand[:])
            nc.vector.match_replace(out=gcand[:], in_to_replace=sl,
                                    in_values=gcand[:], imm_value=SENT)
        emit_tap("d_G", G[:], [B, KE])

        bG_ps = ps.tile([128, KE], f32, tag="psa")
        nc.tensor.matmul(bG_ps[:], lhsT=L4[:], rhs=G[:], start=True,
                         stop=True)
        bG = sb.tile([128, KE], f32)
        nc.vector.tensor_copy(bG[:], bG_ps[:])
        idxu = sb.tile([128, KE], u32)
        for r in range(R3):
            nc.vector.max_index(out=idxu[:, r * 8:(r + 1) * 8],
                                in_max=bG[:, r * 8:(r + 1) * 8],
                                in_values=Worig)
        idxf = sb.tile([128, KE], f32)
        nc.vector.tensor_copy(idxf[:], idxu[:])
        nc.vector.tensor_tensor(idxf[:], idxf[:],
                                p1024[:].to_broadcast([128, KE]), op=Alu.add)
        nc.vector.tensor_tensor(idxf[:], idxf[:],
                                bsub[:].to_broadcast([128, KE]),
                                op=Alu.subtract)
        tidx_ps = ps.tile([KE, 128], f32, tag="psa")
        nc.tensor.transpose(tidx_ps[:], idxf[:], ident[:])
        tidx = sb.tile([KE, 128], f32)
        nc.vector.tensor_copy(tidx[:], tidx_ps[:])
        lminT = sb.tile([KE, B], f32)
        for b in range(B):
            nc.vector.tensor_reduce(lminT[:, b:b + 1],
                                    tidx[:, 32 * b:32 * b + 32],
                                    axis=Axis.X, op=Alu.min)
        emit_tap("d_lminT", lminT[:], [KE, B])

        # own = (local voxel id found), gather rows:
        # sel row: b*VL + v (flat);  he row: b*VL + (v%128)*TDIV + v//128
        own_k = sb.tile([KE, B], f32)
        nc.vector.tensor_scalar(own_k[:], lminT[:], 1.0e6, None,
                                op0=Alu.is_le)
        locv = sb.tile([KE, B], f32)
        nc.vector.tensor_scalar_min(locv[:], lminT[:], float(VL))

        idxs_f = sb.tile([KE, B], f32)
        nc.vector.tensor_tensor(idxs_f[:], locv[:], bcol[:], op=Alu.add)
        notown = sb.tile([KE, B], f32)
        nc.vector.tensor_scalar(notown[:], own_k[:], 0.0, None,
                                op0=Alu.is_equal)
        nc.vector.tensor_scalar_mul(notown[:], notown[:], NOTFOUND_CLAMP)
        nc.vector.tensor_tensor(idxs_f[:], idxs_f[:], notown[:], op=Alu.add)
        nc.vector.tensor_scalar_min(idxs_f[:], idxs_f[:], NOTFOUND_CLAMP)
        idxs_all = sb.tile([KE, B], i32)
        nc.vector.tensor_copy(idxs_all[:], idxs_f[:])

        vdiv = sb.tile([KE, B], f32)
        nc.vector.tensor_scalar(vdiv[:], locv[:], 1.0 / 128.0, None,
                                op0=Alu.mult)
        floor_f(vdiv[:], [KE, B], "fvd")
        vmod = sb.tile([KE, B], f32)
        nc.vector.scalar_tensor_tensor(vmod[:], in0=vdiv[:], scalar=-128.0,
                                       in1=locv[:], op0=Alu.mult,
                                       op1=Alu.add)
        row_f = sb.tile([KE, B], f32)
        nc.vector.scalar_tensor_tensor(row_f[:], in0=vmod[:],
                                       scalar=float(TDIV), in1=vdiv[:],
                                       op0=Alu.mult, op1=Alu.add)
        nc.vector.tensor_tensor(row_f[:], row_f[:], bcol[:], op=Alu.add)
        nc.vector.tensor_tensor(row_f[:], row_f[:], notown[:], op=Alu.add)
        nc.vector.tensor_scalar_min(row_f[:], row_f[:], NOTFOUND_CLAMP)
        idxe_all = sb.tile([KE, B], i32)
        nc.vector.tensor_copy(idxe_all[:], row_f[:])
        emit_tap("d_idxe", row_f[:], [KE, B])

        he_all = sb.tile([KE, B, F], fp8)
        nc.vector.memset(he_all[:], 0.0)
        sel_all = sb.tile([KE, B], f32)
        nc.vector.memset(sel_all[:], 0.0)

        for ci in range(4, NCHUNK):
            stream_chunk(ci)

        # ============ tail: gathers + AllGather#2 (gpsimd FIFO) ============
        embT_flat = embT_dram[:].rearrange("b p t f -> (b p t) f")
        for b in range(B):
            nc.gpsimd.indirect_dma_start(
                out=he_all[:, b, :], out_offset=None,
                in_=embT_flat,
                in_offset=IndirectOffsetOnAxis(ap=idxe_all[:, b:b + 1],
                                               axis=0),
                bounds_check=B * VL - 1, oob_is_err=False)
            nc.gpsimd.indirect_dma_start(
                out=sel_all[:, b:b + 1], out_offset=None,
                in_=s_dram[:],
                in_offset=IndirectOffsetOnAxis(ap=idxs_all[:, b:b + 1],
                                               axis=0),
                bounds_check=B * VL - 1, oob_is_err=False)
        emit_tap("d_he0", he_all[:, 0, :], [KE, F], fp8)
        emit_tap("d_sel0", sel_all[:, 0:1], [KE, 1])

        # class_sum extract: fold the 4 valid [4,64] quadrants via PE
        # (32-aligned partition rule forbids direct offset-partition adds)
        csA_sb = sb.tile([8, 128], bf16)
        nc.vector.tensor_copy(csA_sb[:], csA[:])
        csB_sb = sb.tile([8, 128], bf16)
        nc.vector.tensor_copy(csB_sb[:], csB[:])
        selb0 = ident_bf[0:8, 0:4]      # picks rows j=c
        selb1 = ident_bf[0:8, 4:8]      # picks rows j=4+c
        cls_ps = ps.tile([C, F], f32, tag="psa")
        nc.tensor.matmul(cls_ps[:], lhsT=selb0, rhs=csA_sb[:, 0:F],
                         start=True, stop=False, skip_group_check=True)
        nc.tensor.matmul(cls_ps[:], lhsT=selb1, rhs=csA_sb[:, F:2 * F],
                         start=False, stop=False, skip_group_check=True)
        nc.tensor.matmul(cls_ps[:], lhsT=selb0, rhs=csB_sb[:, 0:F],
                         start=False, stop=False, skip_group_check=True)
        nc.tensor.matmul(cls_ps[:], lhsT=selb1, rhs=csB_sb[:, F:2 * F],
                         start=False, stop=True, skip_group_check=True)
        pack2 = sb.tile([C, F + 1], f32)
        nc.vector.tensor_copy(pack2[:, 0:F], cls_ps[:])
        nc.vector.tensor_copy(pack2[:, F:F + 1], cnt_sb[:])
        emit_tap("d_cs", pack2[:, 0:F], [C, F])
        nc.scalar.dma_start(b2_in[:], pack2[:])
        nc.gpsimd.collective_compute(
            "AllGather", Alu.bypass,
            replica_groups=[list(range(NCORES))],
            ins=[b2_in[:].opt()], outs=[b2_out[:].opt()])

        # ============ EMA -> nar ============
        g2t = sb.tile([NCORES, C * (F + 1)], f32)
        nc.scalar.dma_start(g2t[:], b2_out[:])
        gsum_ps = ps.tile([1, C * (F + 1)], f32, tag="psa")
        nc.tensor.matmul(gsum_ps[:], lhsT=ones8[:], rhs=g2t[:],
                         start=True, stop=True)
        gsum = sb.tile([1, C * (F + 1)], f32)
        nc.vector.tensor_copy(gsum[:], gsum_ps[:])
        emit_tap("d_gsum", gsum[:], [1, C * (F + 1)])
        cs_g = gsum[:].rearrange("a (c f) -> a c f", c=C)[:, :, 0:F]
        cnt_g = gsum[:].rearrange("a (c f) -> a c f", c=C)[:, :, F:F + 1]

        cntc = sb.tile([1, C, 1], f32)
        nc.vector.tensor_scalar_max(cntc[:], cnt_g, 1.0)
        recip = sb.tile([1, C, 1], f32)
        nc.vector.reciprocal(recip[:], cntc[:])
        mean = sb.tile([1, C, F], f32)
        nc.vector.tensor_tensor(mean[:], cs_g,
                                recip[:].to_broadcast([1, C, F]), op=Alu.mult)
        avg_sb = sb.tile([1, C, F], f32)
        nc.scalar.dma_start(avg_sb[:], avg_d)
        cgt = sb.tile([1, C, 1], f32)
        nc.vector.tensor_scalar(cgt[:], cnt_g, 0.0, None, op0=Alu.is_gt)
        t_ema = sb.tile([1, C, F], f32)
        nc.vector.tensor_tensor(t_ema[:], mean[:], avg_sb[:],
                                op=Alu.subtract)
        nc.vector.scalar_tensor_tensor(t_ema[:], in0=t_ema[:],
                                       scalar=EMA_THETA,
                                       in1=cgt[:].to_broadcast([1, C, F]),
                                       op0=Alu.mult, op1=Alu.mult)
        avg_new = sb.tile([1, C, F], f32)
        nc.vector.tensor_tensor(avg_new[:], avg_sb[:], t_ema[:], op=Alu.add)

        narb_ps = ps.tile([KE, (C - 1) * F], f32, tag="psb", bufs=1)
        nc.tensor.matmul(narb_ps[:], lhsT=ones104[:], rhs=avg_new[:, 1:C, :],
                         start=True, stop=True)
        narb = sb.tile([KE, C - 1, F], f32)
        nc.vector.tensor_copy(narb[:], narb_ps[:])
        emit_tap("d_narb", narb[:], [KE, (C - 1) * F])

        # ============ loss epilogue ============
        hef = sb.tile([KE, B, F], f32)
        nc.vector.tensor_copy(hef[:], he_all[:])
        prod = sb.tile([KE, B, C - 1, F], f32)
        nc.vector.tensor_tensor(
            prod[:],
            hef[:].rearrange("k b (o f) -> k b o f", o=1).to_broadcast(
                [KE, B, C - 1, F]),
            narb[:].rearrange("k (o c) f -> k o c f", o=1).to_broadcast(
                [KE, B, C - 1, F]),
            op=Alu.mult)
        ex = sb.tile([KE, B, C - 1, F], f32)
        nc.scalar.activation(ex[:], prod[:], Act.Exp, scale=1.0 / TAU)
        accl = sb.tile([KE, B, F], f32)
        nc.vector.tensor_tensor(accl[:], ex[:, :, 0, :], ex[:, :, 1, :],
                                op=Alu.add)
        nc.vector.tensor_tensor(accl[:], accl[:], ex[:, :, 2, :], op=Alu.add)
        lnv = sb.tile([KE, B, F], f32)
        nc.scalar.activation(lnv[:], accl[:], Act.Ln)
        lsum = sb.tile([KE, B], f32)
        nc.vector.reduce_sum(lsum[:], lnv[:], axis=Axis.X)
        nc.vector.tensor_scalar_mul(lsum[:], lsum[:], -1.0)
        nc.vector.tensor_tensor(lsum[:], lsum[:], sel_all[:], op=Alu.mult)
        losscols = sb.tile([KE, B], f32)
        nc.vector.tensor_tensor(losscols[:], lsum[:], own_k[:], op=Alu.mult)

        final_ps = ps.tile([1, B * KE], f32, tag="psa")
        for b in range(B):
            nc.tensor.matmul(final_ps[:, b * KE:(b + 1) * KE],
                             lhsT=losscols[:, b:b + 1], rhs=ident[:KE, :KE],
                             start=True, stop=True)
        final = sb.tile([1, B * KE], f32)
        nc.vector.tensor_copy(final[:], final_ps[:])
        nc.sync.dma_start(out_d, final[:])


_NC_CACHE = {}


def _get_graph():
    if "nc" not in _NC_CACHE:
        _NC_CACHE["nc"] = build_graph()
    return _NC_CACHE["nc"]


def kernel(proba, y, embeddings, avg_repr):
    from concourse.bass_utils import run_bass_kernel_spmd

    proba = np.asarray(proba, dtype=np.float32)
    y = np.asarray(y, dtype=np.int32)
    embeddings = np.asarray(embeddings, dtype=np.float32)
    avg_repr = np.asarray(avg_repr, dtype=np.float32)

    nc = _get_graph()
    in_maps = []
    for ci in range(NCORES):
        sl = slice(ci * VL, (ci + 1) * VL)
        in_maps.append({
            "proba": np.ascontiguousarray(proba[:, :, sl]),
            "y": np.ascontiguousarray(y[:, :, sl]),
            "embeddings": np.ascontiguousarray(embeddings[:, :, sl]),
            "avg_repr": avg_repr,
        })
    res = run_bass_kernel_spmd(nc, in_maps, core_ids=list(range(NCORES)))
    parts = [res.results[ci]["out"] for ci in range(NCORES)]
    total = np.sum(parts, axis=0).astype(np.float32)
    return total[:, :K]


# revision 23
# speedup vs baseline: 1.2271x; 1.2271x over previous
"""Trainium2 Bass kernel for nn_AnatomicalContrastiveLoss.

Distribution: V (voxel) dim sharded 8 ways; every core holds all B,C,F.
Pipeline per core (engine assignment in brackets):
  - proba -> weights -> local top-32/b [sync DMA + DVE] -> AllGather#1
    fired ~15us in [gpsimd]; merge + index recovery run mid-stream.
  - emb streamed as 8 x [128, 8192] f32 chunks (b-pairs) [sync HWDGE],
    PE-transposed in [128,128] blocks (f32, 2cyc/row), PSUM->SBUF copy
    with fp8 cast split DVE/scalar, fp8 class-sum matmuls accumulate in
    PSUM, fp8 ebT evicted to DRAM in a (v%128)-major layout so each
    eviction is 4KB-contiguous per partition [sync].
  - tail: batched indirect he/sel gathers [gpsimd], AllGather#2 of
    class_sum/count [gpsimd], EMA -> nar -> loss epilogue.
Host combines per-core loss partials (disjoint support), slices [:, :100].
"""

import numpy as np

import concourse.bass as bass
import concourse.tile as tile
from concourse import bacc, mybir
from concourse.bass import IndirectOffsetOnAxis
from concourse.masks import make_identity

B, C, V, F, K = 4, 4, 262144, 64, 100
NCORES = 8
VL = V // NCORES          # 32768
R1 = 3                    # L1 rounds: top-24 per 1024-voxel partition
R2 = 4                    # L2 rounds: top-32 per core
R3 = 13                   # L3 rounds: top-104 global
KE = 8 * R3               # 104 extracted globally
NL = 8 * R2               # 32 local candidates per b
EMA_THETA = 0.9
TAU = 0.1
SENT = -1.0               # removal sentinel (< any weight; weights in (0,1))
NOTFOUND_CLAMP = 2.0e7    # > B*VL, makes unowned gather indices OOB-skipped
CH = 8192                 # emb chunk columns (voxels per chunk)
NCHUNK = 2 * (VL // CH)   # 2 b-pairs x 4 h-slices = 8
TDIV = VL // 128          # 256 rows per vp in sigma layout

f32 = mybir.dt.float32
bf16 = mybir.dt.bfloat16
fp8 = mybir.dt.float8e4
i32 = mybir.dt.int32
u32 = mybir.dt.uint32
Alu = mybir.AluOpType
Act = mybir.ActivationFunctionType
Axis = mybir.AxisListType


def build_graph(debug_taps=False):
    nc = bacc.Bacc("TRN2", target_bir_lowering=False, debug=False,
                   num_devices=NCORES)

    proba_d = nc.dram_tensor("proba", [B, C, VL], f32, kind="ExternalInput")
    y_d = nc.dram_tensor("y", [B, C, VL], i32, kind="ExternalInput")
    emb_d = nc.dram_tensor("embeddings", [B, F, VL], f32, kind="ExternalInput")
    avg_d = nc.dram_tensor("avg_repr", [1, C, F], f32, kind="ExternalInput")
    out_d = nc.dram_tensor("out", [B, KE], f32, kind="ExternalOutput")

    taps = {}

    def tap(name, shape, dtype=f32):
        if not debug_taps:
            return None
        t = nc.dram_tensor(name, shape, dtype, kind="ExternalOutput")
        taps[name] = t
        return t.ap()

    with tile.TileContext(nc) as tc:
        _build(tc, proba_d.ap(), y_d.ap(), emb_d.ap(), avg_d.ap(), out_d.ap(),
               tap)

    nc.compile()
    return nc


def _build(tc, proba_d, y_d, emb_d, avg_d, out_d, tap=lambda *a, **k: None):
    nc = tc.nc

    def emit_tap(name, src_ap, shape, dtype=f32):
        t = tap(name, shape, dtype)
        if t is not None:
            nc.scalar.dma_start(t, src_ap)

    import contextlib
    ctx = contextlib.ExitStack()
    with ctx:
        sb = ctx.enter_context(tc.tile_pool(name="sb", bufs=1))
        sb2 = ctx.enter_context(tc.tile_pool(name="sb2", bufs=2))
        sbch = ctx.enter_context(tc.tile_pool(name="sbch", bufs=2))
        sbt = ctx.enter_context(tc.tile_pool(name="sbt", bufs=2))
        ps = ctx.enter_context(tc.tile_pool(name="ps", bufs=2, space="PSUM"))
        dram = ctx.enter_context(tc.tile_pool(name="dram", bufs=1,
                                              space="DRAM"))

        # ============ DRAM scratch ============
        # (pair, vp, tt, half, f): eviction writes [128,64,128] tiles with
        # 8KB-contiguous per-partition runs; gather row = pair*2*VL//... see
        # row formula at idxe computation.
        embT_dram = dram.tile([2, 128, TDIV, 2, F], fp8)
        s_dram = dram.tile([B * VL, 1], f32)
        b1_in = dram.tile([B, NL], f32)
        b1_out = dram.tile([NCORES, B, NL], f32, addr_space="Shared")
        b2_in = dram.tile([C, F + 1], f32)
        b2_out = dram.tile([NCORES, C, F + 1], f32, addr_space="Shared")

        # ============ input loads (issue order sets priority) ============
        proba_sb = sb.tile([128, C, 1024], f32)
        for c in range(C):
            nc.sync.dma_start(proba_sb[:, c, :], proba_d[:, c, :])
        # y as [(b c s), 4096]: p = 32b + 8c + s, v = s*4096 + col
        # SWDGE cast-load i32 -> bf16 gives the mask directly (y in {0,1})
        mask8 = sb.tile([128, 4096], bf16)
        nc.gpsimd.dma_start(
            mask8[:], y_d.rearrange("b c (s v) -> (b c s) v", s=8))

        chunks = {}

        def load_chunk(ci):
            p, h = ci % 2, ci // 2
            t = sbch.tile([128, CH], f32, tag="ch", name=f"ch{ci}")
            nc.sync.dma_start(
                t[:],
                emb_d[2 * p:2 * p + 2, :,
                      h * CH:(h + 1) * CH].rearrange("b f v -> (b f) v"))
            chunks[ci] = t

        for ci in range(2):
            load_chunk(ci)

        # ============ constants (gpsimd + DVE, overlap loads) ============
        ident = sb.tile([128, 128], f32)
        make_identity(nc, ident[:])
        ident_bf = sb.tile([128, 128], bf16)
        nc.vector.tensor_copy(ident_bf[:], ident[:])

        iota_p = sb.tile([128, 1], i32)
        nc.gpsimd.iota(iota_p[:], pattern=[[1, 1]], base=0,
                       channel_multiplier=1)
        iota_pf = sb.tile([128, 1], f32)
        nc.vector.tensor_copy(iota_pf[:], iota_p[:])

        def floor_f(x_ap, shape, tg):
            # in-place floor: f32->i32 cast ROUNDS to nearest, so correct
            # by subtracting 1 where the rounded value exceeds the input
            xi = sb2.tile(shape, i32, tag=tg + "i")
            nc.vector.tensor_copy(xi[:], x_ap)
            xr = sb2.tile(shape, f32, tag=tg + "r")
            nc.vector.tensor_copy(xr[:], xi[:])
            corr = sb2.tile(shape, f32, tag=tg + "c")
            nc.vector.tensor_tensor(corr[:], xr[:], x_ap, op=Alu.is_gt)
            nc.vector.tensor_tensor(x_ap, xr[:], corr[:], op=Alu.subtract)

        # b(p) = p//32, s8(p) = p%8
        b_of_p = sb.tile([128, 1], f32)
        nc.vector.tensor_scalar(b_of_p[:], iota_pf[:], 1.0 / 32.0, None,
                                op0=Alu.mult)
        floor_f(b_of_p[:], [128, 1], "fb")
        r32_of_p = sb.tile([128, 1], f32)
        nc.vector.scalar_tensor_tensor(r32_of_p[:], in0=b_of_p[:],
                                       scalar=-32.0, in1=iota_pf[:],
                                       op0=Alu.mult, op1=Alu.add)
        q8_of_p = sb.tile([128, 1], f32)
        nc.vector.tensor_scalar(q8_of_p[:], iota_pf[:], 1.0 / 8.0, None,
                                op0=Alu.mult)
        floor_f(q8_of_p[:], [128, 1], "fq")
        s8_of_p = sb.tile([128, 1], f32)
        nc.vector.scalar_tensor_tensor(s8_of_p[:], in0=q8_of_p[:],
                                       scalar=-8.0, in1=iota_pf[:],
                                       op0=Alu.mult, op1=Alu.add)

        # L4[b, j] = 1 iff j//32 == b
        col128_i = sb.tile([B, 128], i32)
        nc.gpsimd.iota(col128_i[:], pattern=[[1, 128]], base=0,
                       channel_multiplier=0)
        col128 = sb.tile([B, 128], f32)
        nc.vector.tensor_copy(col128[:], col128_i[:])
        bcol4 = sb.tile([B, 1], i32)
        nc.gpsimd.iota(bcol4[:], pattern=[[1, 1]], base=0,
                       channel_multiplier=1)
        bcol4f = sb.tile([B, 1], f32)
        nc.vector.tensor_copy(bcol4f[:], bcol4[:])
        j32 = sb.tile([B, 128], f32)
        nc.vector.tensor_scalar(j32[:], col128[:], 1.0 / 32.0, None,
                                op0=Alu.mult)
        floor_f(j32[:], [B, 128], "fl4")
        L4 = sb.tile([B, 128], f32)
        nc.vector.tensor_tensor(L4[:], j32[:],
                                bcol4f[:].to_broadcast([B, 128]),
                                op=Alu.is_equal)

        # S4[p, j] = 1 iff j == 8*(p//32) + p%8
        col32_i = sb.tile([128, 32], i32)
        nc.gpsimd.iota(col32_i[:], pattern=[[1, 32]], base=0,
                       channel_multiplier=0)
        col32 = sb.tile([128, 32], f32)
        nc.vector.tensor_copy(col32[:], col32_i[:])
        jtgt = sb.tile([128, 1], f32)
        nc.vector.scalar_tensor_tensor(jtgt[:], in0=b_of_p[:], scalar=8.0,
                                       in1=s8_of_p[:], op0=Alu.mult,
                                       op1=Alu.add)
        S_CAT = sb.tile([128, 64], bf16)
        nc.vector.tensor_tensor(S_CAT[:, 0:32], col32[:],
                                jtgt[:].to_broadcast([128, 32]),
                                op=Alu.is_equal)

        # G2[p, c] = 1 iff c == (p%32)//8
        colC_i = sb.tile([128, C], i32)
        nc.gpsimd.iota(colC_i[:], pattern=[[1, C]], base=0,
                       channel_multiplier=0)
        colC = sb.tile([128, C], f32)
        nc.vector.tensor_copy(colC[:], colC_i[:])
        c_of_p = sb.tile([128, 1], f32)
        nc.vector.tensor_scalar(c_of_p[:], r32_of_p[:], 1.0 / 8.0, None,
                                op0=Alu.mult)
        floor_f(c_of_p[:], [128, 1], "fg2")
        G2 = sb.tile([128, C], f32)
        nc.vector.tensor_tensor(G2[:], colC[:],
                                c_of_p[:].to_broadcast([128, C]),
                                op=Alu.is_equal)
        # S0 = S4 masked to c==0 rows: S_CAT[:, 32:64]
        c0ind = sb.tile([128, 1], f32)
        nc.vector.tensor_scalar(c0ind[:], c_of_p[:], 0.0, None,
                                op0=Alu.is_equal)
        nc.vector.tensor_tensor(S_CAT[:, 32:64], S_CAT[:, 0:32],
                                c0ind[:].to_broadcast([128, 32]),
                                op=Alu.mult)

        ones8 = sb.tile([NCORES, 1], f32)
        nc.gpsimd.memset(ones8[:], 1.0)
        ones104 = sb.tile([1, KE], f32)
        nc.gpsimd.memset(ones104[:], 1.0)
        p1024 = sb.tile([128, 1], f32)
        nc.vector.tensor_scalar(p1024[:], iota_pf[:], 1024.0, None,
                                op0=Alu.mult)
        bsub = sb.tile([128, 1], f32)
        nc.vector.tensor_scalar(bsub[:], b_of_p[:], float(VL), None,
                                op0=Alu.mult)
        # bcol[k, b] = b*VL
        bcol_i = sb.tile([KE, B], i32)
        nc.gpsimd.iota(bcol_i[:], pattern=[[1, B]], base=0,
                       channel_multiplier=0)
        bcol = sb.tile([KE, B], f32)
        nc.vector.tensor_copy(bcol[:], bcol_i[:])
        nc.vector.tensor_scalar_mul(bcol[:], bcol[:], float(VL))
        # bcol2[k, b] = (b//2)*(2*VL) + b%2  (he-gather row base)
        bcol2 = sb.tile([KE, B], f32)
        nc.vector.tensor_copy(bcol2[:], bcol_i[:])
        nc.vector.tensor_scalar_mul(bcol2[:], bcol2[:], 0.5)
        floor_f(bcol2[:], [KE, B], "fbc2")
        halfb = sb.tile([KE, B], f32)
        nc.vector.tensor_copy(halfb[:], bcol_i[:])
        nc.vector.scalar_tensor_tensor(halfb[:], in0=bcol2[:], scalar=-2.0,
                                       in1=halfb[:], op0=Alu.mult,
                                       op1=Alu.add)
        nc.vector.tensor_scalar_mul(bcol2[:], bcol2[:], float(2 * VL))
        nc.vector.tensor_tensor(bcol2[:], bcol2[:], halfb[:], op=Alu.add)

        # ============ candidate path (DVE) ============
        # Worig/Wx live inside proba_sb (c=0 / c=1 slots) to save SBUF
        Worig = proba_sb[:, 0, :]
        nc.vector.tensor_tensor(Worig, proba_sb[:, 0, :],
                                proba_sb[:, 1, :], op=Alu.mult)
        nc.vector.tensor_tensor(Worig, Worig, proba_sb[:, 2, :],
                                op=Alu.mult)
        nc.vector.tensor_tensor(Worig, Worig, proba_sb[:, 3, :],
                                op=Alu.mult)
        Wx = proba_sb[:, 1, :]
        nc.vector.tensor_copy(Wx, proba_sb[:, 0, :])
        V1 = sb.tile([128, 8 * R1], f32)
        for r in range(R1):
            sl = V1[:, r * 8:(r + 1) * 8]
            nc.vector.max(out=sl, in_=Wx)
            nc.vector.match_replace(out=Wx, in_to_replace=sl,
                                    in_values=Wx, imm_value=SENT)
        cand = sb.tile([B, 32 * 8 * R1], f32)
        nc.scalar.dma_start(cand[:], V1[:])
        L2V = sb.tile([B, NL], f32)
        for r in range(R2):
            sl = L2V[:, r * 8:(r + 1) * 8]
            nc.vector.max(out=sl, in_=cand[:])
            nc.vector.match_replace(out=cand[:], in_to_replace=sl,
                                    in_values=cand[:], imm_value=SENT)
        emit_tap("d_L2V", L2V[:], [B, NL])
        nc.gpsimd.dma_start(b1_in[:], L2V[:])
        # AllGather #1 — fired early; blocks gpsimd until done (~45us);
        # nothing else is queued on gpsimd until gcand/tail.
        nc.gpsimd.collective_compute(
            "AllGather", Alu.bypass,
            replica_groups=[list(range(NCORES))],
            ins=[b1_in[:].opt()], outs=[b1_out[:].opt()])

        # ============ mask path (fp8) ============
        maskT = sb.tile([128, 32, 128], fp8)
        for g in range(8):
            mt_ps = ps.tile([128, 512], bf16, tag="pstr", bufs=3,
                            name=f"mt{g}")
            for q in range(4):
                s = g * 4 + q
                nc.tensor.transpose(mt_ps[:, q * 128:(q + 1) * 128],
                                    mask8[:, s * 128:(s + 1) * 128],
                                    ident_bf[:])
            nc.vector.tensor_copy(maskT[:, g * 4:(g + 1) * 4, :], mt_ps[:])
        emit_tap("d_maskT", maskT[:, 0, :], [128, 128], fp8)

        cnt_part = sb.tile([128, 1], f32)
        nc.vector.reduce_sum(cnt_part[:], mask8[:], axis=Axis.X)
        cnt_ps = ps.tile([C, 1], f32, tag="psa")
        nc.tensor.matmul(cnt_ps[:], lhsT=G2[:], rhs=cnt_part[:],
                         start=True, stop=True)
        cnt_sb = sb.tile([C, 1], f32)
        nc.vector.tensor_copy(cnt_sb[:], cnt_ps[:])
        emit_tap("d_cnt", cnt_sb[:], [C, 1])

        # sel field: s[b,v] = 1 if (no class active) or (class0 active)
        # one matmul computes [msum | m0] stacked on 64 psum partitions
        sval = sb.tile([32, 4096], f32)
        for q in range(8):
            ms_ps = ps.tile([64, 512], f32, tag="psb", bufs=1,
                            name=f"ms{q}")
            nc.tensor.matmul(ms_ps[:], lhsT=S_CAT[:],
                             rhs=mask8[:, 512 * q:512 * (q + 1)],
                             start=True, stop=True)
            m0sb = sb2.tile([32, 512], f32, tag="m0sb")
            nc.vector.tensor_copy(m0sb[:], ms_ps[32:64, :])
            nc.vector.scalar_tensor_tensor(
                sval[:, 512 * q:512 * (q + 1)], in0=ms_ps[0:32, :],
                scalar=0.0, in1=m0sb[:], op0=Alu.is_equal,
                op1=Alu.max)
        nc.scalar.dma_start(
            s_dram[:].rearrange("(p v) a -> p (v a)", p=32), sval[:])

        # ============ emb streaming ============
        csA = ps.tile([8, 128], f32, tag="pscsA", bufs=1)
        csB = ps.tile([8, 128], f32, tag="pscsB", bufs=1)
        first_mm = {0: True, 1: True}
        n_mm = {0: 0, 1: 0}
        copy_flip = [0]

        def stream_chunk(ci):
            p, h = ci % 2, ci // 2
            t = chunks.pop(ci)
            ebT = sbt.tile([128, 64, 128], fp8, tag="ebT", name=f"ebT{ci}")
            cs_psum = csA if p == 0 else csB
            for g in range(16):
                tp_ps = ps.tile([128, 512], f32, tag="pstr", bufs=3,
                                name=f"tp{ci}_{g}")
                for q in range(4):
                    vb = g * 4 + q
                    nc.tensor.transpose(tp_ps[:, q * 128:(q + 1) * 128],
                                        t[:, vb * 128:(vb + 1) * 128],
                                        ident[:])
                dst = ebT[:, g * 4:(g + 1) * 4, :]
                if copy_flip[0] % 2 == 0:
                    nc.vector.tensor_copy(dst, tp_ps[:])
                else:
                    nc.scalar.activation(dst, tp_ps[:], Act.Copy)
                copy_flip[0] += 1
                for q in range(4):
                    vb = g * 4 + q
                    v0 = h * CH + vb * 128
                    s8, s = v0 // 4096, (v0 % 4096) // 128
                    c0 = s8 + 64 * p
                    lhsT = maskT[:, s, c0:c0 + 57:8]  # 8 cols: pair's (b,c)
                    n_mm[p] += 1
                    nc.tensor.matmul(
                        cs_psum[:], lhsT=lhsT,
                        rhs=ebT[:, vb, :],
                        start=first_mm[p],
                        stop=(n_mm[p] == (VL // 128)),
                        skip_group_check=True)
                    first_mm[p] = False
            # eviction: one DMA, 8KB contiguous per partition both sides
            nc.sync.dma_start(
                embT_dram[p, :, h * 64:(h + 1) * 64, :, :],
                ebT[:].rearrange("vp t (hf f) -> vp t hf f", hf=2))
            if ci + 2 < NCHUNK:
                load_chunk(ci + 2)

        for ci in range(4):
            stream_chunk(ci)

        # ===== merge + index recovery (mid-stream; gcand via gpsimd) =====
        gcand = sb.tile([B, NCORES * NL], f32)
        nc.gpsimd.dma_start(
            gcand[:].rearrange("b (ci r) -> b ci r", ci=NCORES),
            b1_out[:].rearrange("ci b r -> b ci r"))
        emit_tap("d_gcand", gcand[:], [B, NCORES * NL])
        G = sb.tile([B, KE], f32)
        for r in range(R3):
            sl = G[:, r * 8:(r + 1) * 8]
            nc.vector.max(out=sl, in_=gcand[:])
            nc.vector.match_replace(out=gcand[:], in_to_replace=sl,
                                    in_values=gcand[:], imm_value=SENT)
        emit_tap("d_G", G[:], [B, KE])

        bG_ps = ps.tile([128, KE], f32, tag="psa")
        nc.tensor.matmul(bG_ps[:], lhsT=L4[:], rhs=G[:], start=True,
                         stop=True)
        bG = sb.tile([128, KE], f32)
        nc.vector.tensor_copy(bG[:], bG_ps[:])
        idxu = sb.tile([128, KE], u32)
        for r in range(R3):
            nc.vector.max_index(out=idxu[:, r * 8:(r + 1) * 8],
                                in_max=bG[:, r * 8:(r + 1) * 8],
                                in_values=Worig)
        idxf = sb.tile([128, KE], f32)
        nc.vector.tensor_copy(idxf[:], idxu[:])
        nc.vector.tensor_tensor(idxf[:], idxf[:],
                                p1024[:].to_broadcast([128, KE]), op=Alu.add)
        nc.vector.tensor_tensor(idxf[:], idxf[:],
                                bsub[:].to_broadcast([128, KE]),
                                op=Alu.subtract)
        tidx_ps = ps.tile([KE, 128], f32, tag="psa")
        nc.tensor.transpose(tidx_ps[:], idxf[:], ident[:])
        tidx = sb.tile([KE, 128], f32)
        nc.vector.tensor_copy(tidx[:], tidx_ps[:])
        lminT = sb.tile([KE, B], f32)
        for b in range(B):
            nc.vector.tensor_reduce(lminT[:, b:b + 1],
                                    tidx[:, 32 * b:32 * b + 32],
                                    axis=Axis.X, op=Alu.min)
        emit_tap("d_lminT", lminT[:], [KE, B])

        # own = (local voxel id found), gather rows:
        # sel row: b*VL + v (flat);  he row: b*VL + (v%128)*TDIV + v//128
        own_k = sb.tile([KE, B], f32)
        nc.vector.tensor_scalar(own_k[:], lminT[:], 1.0e6, None,
                                op0=Alu.is_le)
        locv = sb.tile([KE, B], f32)
        nc.vector.tensor_scalar_min(locv[:], lminT[:], float(VL))

        idxs_f = sb.tile([KE, B], f32)
        nc.vector.tensor_tensor(idxs_f[:], locv[:], bcol[:], op=Alu.add)
        notown = sb.tile([KE, B], f32)
        nc.vector.tensor_scalar(notown[:], own_k[:], 0.0, None,
                                op0=Alu.is_equal)
        nc.vector.tensor_scalar_mul(notown[:], notown[:], NOTFOUND_CLAMP)
        nc.vector.tensor_tensor(idxs_f[:], idxs_f[:], notown[:], op=Alu.add)
        nc.vector.tensor_scalar_min(idxs_f[:], idxs_f[:], NOTFOUND_CLAMP)
        idxs_all = sb.tile([KE, B], i32)
        nc.vector.tensor_copy(idxs_all[:], idxs_f[:])

        vdiv = sb.tile([KE, B], f32)
        nc.vector.tensor_scalar(vdiv[:], locv[:], 1.0 / 128.0, None,
                                op0=Alu.mult)
        floor_f(vdiv[:], [KE, B], "fvd")
        vmod = sb.tile([KE, B], f32)
        nc.vector.scalar_tensor_tensor(vmod[:], in0=vdiv[:], scalar=-128.0,
                                       in1=locv[:], op0=Alu.mult,
                                       op1=Alu.add)
        row_f = sb.tile([KE, B], f32)
        nc.vector.scalar_tensor_tensor(row_f[:], in0=vdiv[:], scalar=2.0,
                                       in1=bcol2[:], op0=Alu.mult,
                                       op1=Alu.add)
        nc.vector.scalar_tensor_tensor(row_f[:], in0=vmod[:],
                                       scalar=float(2 * TDIV), in1=row_f[:],
                                       op0=Alu.mult, op1=Alu.add)
        nc.vector.tensor_tensor(row_f[:], row_f[:], notown[:], op=Alu.add)
        nc.vector.tensor_scalar_min(row_f[:], row_f[:], NOTFOUND_CLAMP)
        idxe_all = sb.tile([KE, B], i32)
        nc.vector.tensor_copy(idxe_all[:], row_f[:])
        emit_tap("d_idxe", row_f[:], [KE, B])

        he_all = sb.tile([KE, B, F], fp8)
        nc.vector.memset(he_all[:], 0.0)
        sel_all = sb.tile([KE, B], f32)
        nc.vector.memset(sel_all[:], 0.0)

        for ci in range(4, NCHUNK):
            stream_chunk(ci)

        # ============ tail: gathers + AllGather#2 (gpsimd FIFO) ============
        embT_flat = embT_dram[:].rearrange("q p t hf f -> (q p t hf) f")
        for b in range(B):
            nc.gpsimd.indirect_dma_start(
                out=he_all[:, b, :], out_offset=None,
                in_=embT_flat,
                in_offset=IndirectOffsetOnAxis(ap=idxe_all[:, b:b + 1],
                                               axis=0),
                bounds_check=B * VL - 1, oob_is_err=False)
            nc.gpsimd.indirect_dma_start(
                out=sel_all[:, b:b + 1], out_offset=None,
                in_=s_dram[:],
                in_offset=IndirectOffsetOnAxis(ap=idxs_all[:, b:b + 1],
                                               axis=0),
                bounds_check=B * VL - 1, oob_is_err=False)
        emit_tap("d_he0", he_all[:, 0, :], [KE, F], fp8)
        emit_tap("d_sel0", sel_all[:, 0:1], [KE, 1])

        # class_sum extract: fold the 4 valid [4,64] quadrants via PE
        # (32-aligned partition rule forbids direct offset-partition adds)
        csA_sb = sb.tile([8, 128], bf16)
        nc.vector.tensor_copy(csA_sb[:], csA[:])
        csB_sb = sb.tile([8, 128], bf16)
        nc.vector.tensor_copy(csB_sb[:], csB[:])
        selb0 = ident_bf[0:8, 0:4]      # picks rows j=c
        selb1 = ident_bf[0:8, 4:8]      # picks rows j=4+c
        cls_ps = ps.tile([C, F], f32, tag="psa")
        nc.tensor.matmul(cls_ps[:], lhsT=selb0, rhs=csA_sb[:, 0:F],
                         start=True, stop=False, skip_group_check=True)
        nc.tensor.matmul(cls_ps[:], lhsT=selb1, rhs=csA_sb[:, F:2 * F],
                         start=False, stop=False, skip_group_check=True)
        nc.tensor.matmul(cls_ps[:], lhsT=selb0, rhs=csB_sb[:, 0:F],
                         start=False, stop=False, skip_group_check=True)
        nc.tensor.matmul(cls_ps[:], lhsT=selb1, rhs=csB_sb[:, F:2 * F],
                         start=False, stop=True, skip_group_check=True)
        pack2 = sb.tile([C, F + 1], f32)
        nc.vector.tensor_copy(pack2[:, 0:F], cls_ps[:])
        nc.vector.tensor_copy(pack2[:, F:F + 1], cnt_sb[:])
        emit_tap("d_cs", pack2[:, 0:F], [C, F])
        nc.scalar.dma_start(b2_in[:], pack2[:])
        nc.gpsimd.collective_compute(
            "AllGather", Alu.bypass,
            replica_groups=[list(range(NCORES))],
            ins=[b2_in[:].opt()], outs=[b2_out[:].opt()])

        # ============ EMA -> nar ============
        g2t = sb.tile([NCORES, C * (F + 1)], f32)
        nc.scalar.dma_start(g2t[:], b2_out[:])
        gsum_ps = ps.tile([1, C * (F + 1)], f32, tag="psa")
        nc.tensor.matmul(gsum_ps[:], lhsT=ones8[:], rhs=g2t[:],
                         start=True, stop=True)
        gsum = sb.tile([1, C * (F + 1)], f32)
        nc.vector.tensor_copy(gsum[:], gsum_ps[:])
        emit_tap("d_gsum", gsum[:], [1, C * (F + 1)])
        cs_g = gsum[:].rearrange("a (c f) -> a c f", c=C)[:, :, 0:F]
        cnt_g = gsum[:].rearrange("a (c f) -> a c f", c=C)[:, :, F:F + 1]

        cntc = sb.tile([1, C, 1], f32)
        nc.vector.tensor_scalar_max(cntc[:], cnt_g, 1.0)
        recip = sb.tile([1, C, 1], f32)
        nc.vector.reciprocal(recip[:], cntc[:])
        mean = sb.tile([1, C, F], f32)
        nc.vector.tensor_tensor(mean[:], cs_g,
                                recip[:].to_broadcast([1, C, F]), op=Alu.mult)
        avg_sb = sb.tile([1, C, F], f32)
        nc.scalar.dma_start(avg_sb[:], avg_d)
        cgt = sb.tile([1, C, 1], f32)
        nc.vector.tensor_scalar(cgt[:], cnt_g, 0.0, None, op0=Alu.is_gt)
        t_ema = sb.tile([1, C, F], f32)
        nc.vector.tensor_tensor(t_ema[:], mean[:], avg_sb[:],
                                op=Alu.subtract)
        nc.vector.scalar_tensor_tensor(t_ema[:], in0=t_ema[:],
                                       scalar=EMA_THETA,
                                       in1=cgt[:].to_broadcast([1, C, F]),
                                       op0=Alu.mult, op1=Alu.mult)
        avg_new = sb.tile([1, C, F], f32)
        nc.vector.tensor_tensor(avg_new[:], avg_sb[:], t_ema[:], op=Alu.add)

        narb_ps = ps.tile([KE, (C - 1) * F], f32, tag="psb", bufs=1)
        nc.tensor.matmul(narb_ps[:], lhsT=ones104[:], rhs=avg_new[:, 1:C, :],
                         start=True, stop=True)
        narb = sb.tile([KE, C - 1, F], f32)
        nc.vector.tensor_copy(narb[:], narb_ps[:])
        emit_tap("d_narb", narb[:], [KE, (C - 1) * F])

        # ============ loss epilogue ============
        hef = sb.tile([KE, B, F], f32)
        nc.vector.tensor_copy(hef[:], he_all[:])
        prod = sb.tile([KE, B, C - 1, F], f32)
        nc.vector.tensor_tensor(
            prod[:],
            hef[:].rearrange("k b (o f) -> k b o f", o=1).to_broadcast(
                [KE, B, C - 1, F]),
            narb[:].rearrange("k (o c) f -> k o c f", o=1).to_broadcast(
                [KE, B, C - 1, F]),
            op=Alu.mult)
        ex = sb.tile([KE, B, C - 1, F], f32)
        nc.scalar.activation(ex[:], prod[:], Act.Exp, scale=1.0 / TAU)
        accl = sb.tile([KE, B, F], f32)
        nc.vector.tensor_tensor(accl[:], ex[:, :, 0, :], ex[:, :, 1, :],
                                op=Alu.add)
        nc.vector.tensor_tensor(accl[:], accl[:], ex[:, :, 2, :], op=Alu.add)
        lnv = sb.tile([KE, B, F], f32)
        nc.scalar.activation(lnv[:], accl[:], Act.Ln)
        lsum = sb.tile([KE, B], f32)
        nc.vector.reduce_sum(lsum[:], lnv[:], axis=Axis.X)
        nc.vector.tensor_scalar_mul(lsum[:], lsum[:], -1.0)
        nc.vector.tensor_tensor(lsum[:], lsum[:], sel_all[:], op=Alu.mult)
        losscols = sb.tile([KE, B], f32)
        nc.vector.tensor_tensor(losscols[:], lsum[:], own_k[:], op=Alu.mult)

        final_ps = ps.tile([1, B * KE], f32, tag="psa")
        for b in range(B):
            nc.tensor.matmul(final_ps[:, b * KE:(b + 1) * KE],
                             lhsT=losscols[:, b:b + 1], rhs=ident[:KE, :KE],
                             start=True, stop=True)
        final = sb.tile([1, B * KE], f32)
        nc.vector.tensor_copy(final[:], final_ps[:])
        nc.sync.dma_start(out_d, final[:])


_NC_CACHE = {}


def _get_graph():
    if "nc" not in _NC_CACHE:
        _NC_CACHE["nc"] = build_graph()
    return _NC_CACHE["nc"]


def kernel(proba, y, embeddings, avg_repr):
    from concourse.bass_utils import run_bass_kernel_spmd

    proba = np.asarray(proba, dtype=np.float32)
    y = np.asarray(y, dtype=np.int32)
    embeddings = np.asarray(embeddings, dtype=np.float32)
    avg_repr = np.asarray(avg_repr, dtype=np.float32)

    nc = _get_graph()
    in_maps = []
    for ci in range(NCORES):
        sl = slice(ci * VL, (ci + 1) * VL)
        in_maps.append({
            "proba": np.ascontiguousarray(proba[:, :, sl]),
            "y": np.ascontiguousarray(y[:, :, sl]),
            "embeddings": np.ascontiguousarray(embeddings[:, :, sl]),
            "avg_repr": avg_repr,
        })
    res = run_bass_kernel_spmd(nc, in_maps, core_ids=list(range(NCORES)))
    parts = [res.results[ci]["out"] for ci in range(NCORES)]
    total = np.sum(parts, axis=0).astype(np.float32)
    return total[:, :K]


# revision 26
# speedup vs baseline: 1.4042x; 1.1443x over previous
"""Trainium2 Bass kernel for nn_AnatomicalContrastiveLoss.

Distribution: V (voxel) dim sharded 8 ways; every core holds all B,C,F.
Pipeline per core (engine assignment in brackets):
  - proba -> weights -> local top-32/b [sync DMA + DVE] -> AllGather#1
    fired ~15us in [gpsimd]; merge + index recovery run mid-stream.
  - emb streamed as 8 x [128, 8192] f32 chunks (b-pairs) [sync HWDGE],
    PE-transposed in [128,128] blocks (f32, 2cyc/row), PSUM->SBUF copy
    with fp8 cast split DVE/scalar, fp8 class-sum matmuls accumulate in
    PSUM, fp8 ebT evicted to DRAM in a (v%128)-major layout so each
    eviction is 4KB-contiguous per partition [sync].
  - tail: batched indirect he/sel gathers [gpsimd], AllGather#2 of
    class_sum/count [gpsimd], EMA -> nar -> loss epilogue.
Host combines per-core loss partials (disjoint support), slices [:, :100].
"""

import numpy as np

import concourse.bass as bass
import concourse.tile as tile
from concourse import bacc, mybir
from concourse.bass import IndirectOffsetOnAxis
from concourse.masks import make_identity

B, C, V, F, K = 4, 4, 262144, 64, 100
NCORES = 8
VL = V // NCORES          # 32768
R1 = 3                    # L1 rounds: top-24 per 1024-voxel partition
R2 = 4                    # L2 rounds: top-32 per core
R3 = 13                   # L3 rounds: top-104 global
KE = 8 * R3               # 104 extracted globally
NL = 8 * R2               # 32 local candidates per b
EMA_THETA = 0.9
TAU = 0.1
SENT = -1.0               # removal sentinel (< any weight; weights in (0,1))
NOTFOUND_CLAMP = 2.0e7    # > B*VL, makes unowned gather indices OOB-skipped
CH = 8192                 # emb chunk columns (voxels per chunk)
NCHUNK = 2 * (VL // CH)   # 2 b-pairs x 4 h-slices = 8
TDIV = VL // 128          # 256 rows per vp in sigma layout

f32 = mybir.dt.float32
bf16 = mybir.dt.bfloat16
fp8 = mybir.dt.float8e4
i32 = mybir.dt.int32
u32 = mybir.dt.uint32
Alu = mybir.AluOpType
Act = mybir.ActivationFunctionType
Axis = mybir.AxisListType


def build_graph(debug_taps=False):
    nc = bacc.Bacc("TRN2", target_bir_lowering=False, debug=False,
                   num_devices=NCORES)

    proba_d = nc.dram_tensor("proba", [B, C, VL], f32, kind="ExternalInput")
    y_d = nc.dram_tensor("y", [B, C, VL], i32, kind="ExternalInput")
    emb_d = nc.dram_tensor("embeddings", [B, F, VL], f32, kind="ExternalInput")
    avg_d = nc.dram_tensor("avg_repr", [1, C, F], f32, kind="ExternalInput")
    out_d = nc.dram_tensor("out", [B, KE], f32, kind="ExternalOutput")

    taps = {}

    def tap(name, shape, dtype=f32):
        if not debug_taps:
            return None
        t = nc.dram_tensor(name, shape, dtype, kind="ExternalOutput")
        taps[name] = t
        return t.ap()

    with tile.TileContext(nc) as tc:
        _build(tc, proba_d.ap(), y_d.ap(), emb_d.ap(), avg_d.ap(), out_d.ap(),
               tap)

    nc.compile()
    return nc


def _build(tc, proba_d, y_d, emb_d, avg_d, out_d, tap=lambda *a, **k: None):
    nc = tc.nc

    def emit_tap(name, src_ap, shape, dtype=f32):
        t = tap(name, shape, dtype)
        if t is not None:
            nc.scalar.dma_start(t, src_ap)

    import contextlib
    ctx = contextlib.ExitStack()
    with ctx:
        sb = ctx.enter_context(tc.tile_pool(name="sb", bufs=1))
        sb2 = ctx.enter_context(tc.tile_pool(name="sb2", bufs=2))
        sbch = ctx.enter_context(tc.tile_pool(name="sbch", bufs=2))
        sbt = ctx.enter_context(tc.tile_pool(name="sbt", bufs=2))
        ps = ctx.enter_context(tc.tile_pool(name="ps", bufs=2, space="PSUM"))
        dram = ctx.enter_context(tc.tile_pool(name="dram", bufs=1,
                                              space="DRAM"))

        # ============ DRAM scratch ============
        # (pair, vp, tt, half, f): eviction writes [128,64,128] tiles with
        # 8KB-contiguous per-partition runs; gather row = pair*2*VL//... see
        # row formula at idxe computation.
        embT_dram = dram.tile([2, 128, TDIV, 2, F], fp8)
        s_dram = dram.tile([B * VL, 1], f32)
        b1_in = dram.tile([B, NL], f32)
        b1_out = dram.tile([NCORES, B, NL], f32, addr_space="Shared")
        b2_in = dram.tile([C, F + 1], f32)
        b2_out = dram.tile([NCORES, C, F + 1], f32, addr_space="Shared")

        # ============ input loads (issue order sets priority) ============
        proba_sb = sb.tile([128, C, 1024], f32)
        for c in range(C):
            nc.sync.dma_start(proba_sb[:, c, :], proba_d[:, c, :])
        # y as [(b c s), 4096]: p = 32b + 8c + s, v = s*4096 + col
        # SWDGE cast-load i32 -> bf16 gives the mask directly (y in {0,1})
        mask8 = sb.tile([128, 4096], bf16)
        nc.gpsimd.dma_start(
            mask8[:], y_d.rearrange("b c (s v) -> (b c s) v", s=8))

        chunks = {}

        def load_chunk(ci):
            p, h = ci % 2, ci // 2
            t = sbch.tile([128, CH], f32, tag="ch", name=f"ch{ci}")
            nc.sync.dma_start(
                t[:],
                emb_d[2 * p:2 * p + 2, :,
                      h * CH:(h + 1) * CH].rearrange("b f v -> (b f) v"))
            chunks[ci] = t

        for ci in range(2):
            load_chunk(ci)

        # ============ constants (gpsimd + DVE, overlap loads) ============
        ident = sb.tile([128, 128], f32)
        make_identity(nc, ident[:])
        ident_bf = sb.tile([128, 128], bf16)
        nc.vector.tensor_copy(ident_bf[:], ident[:])

        iota_p = sb.tile([128, 1], i32)
        nc.gpsimd.iota(iota_p[:], pattern=[[1, 1]], base=0,
                       channel_multiplier=1)
        iota_pf = sb.tile([128, 1], f32)
        nc.vector.tensor_copy(iota_pf[:], iota_p[:])

        def floor_f(x_ap, shape, tg):
            # in-place floor: f32->i32 cast ROUNDS to nearest, so correct
            # by subtracting 1 where the rounded value exceeds the input
            xi = sb2.tile(shape, i32, tag=tg + "i")
            nc.vector.tensor_copy(xi[:], x_ap)
            xr = sb2.tile(shape, f32, tag=tg + "r")
            nc.vector.tensor_copy(xr[:], xi[:])
            corr = sb2.tile(shape, f32, tag=tg + "c")
            nc.vector.tensor_tensor(corr[:], xr[:], x_ap, op=Alu.is_gt)
            nc.vector.tensor_tensor(x_ap, xr[:], corr[:], op=Alu.subtract)

        # b(p) = p//32, s8(p) = p%8
        b_of_p = sb.tile([128, 1], f32)
        nc.vector.tensor_scalar(b_of_p[:], iota_pf[:], 1.0 / 32.0, None,
                                op0=Alu.mult)
        floor_f(b_of_p[:], [128, 1], "fb")
        r32_of_p = sb.tile([128, 1], f32)
        nc.vector.scalar_tensor_tensor(r32_of_p[:], in0=b_of_p[:],
                                       scalar=-32.0, in1=iota_pf[:],
                                       op0=Alu.mult, op1=Alu.add)
        q8_of_p = sb.tile([128, 1], f32)
        nc.vector.tensor_scalar(q8_of_p[:], iota_pf[:], 1.0 / 8.0, None,
                                op0=Alu.mult)
        floor_f(q8_of_p[:], [128, 1], "fq")
        s8_of_p = sb.tile([128, 1], f32)
        nc.vector.scalar_tensor_tensor(s8_of_p[:], in0=q8_of_p[:],
                                       scalar=-8.0, in1=iota_pf[:],
                                       op0=Alu.mult, op1=Alu.add)

        # L4[b, j] = 1 iff j//32 == b
        col128_i = sb.tile([B, 128], i32)
        nc.gpsimd.iota(col128_i[:], pattern=[[1, 128]], base=0,
                       channel_multiplier=0)
        col128 = sb.tile([B, 128], f32)
        nc.vector.tensor_copy(col128[:], col128_i[:])
        bcol4 = sb.tile([B, 1], i32)
        nc.gpsimd.iota(bcol4[:], pattern=[[1, 1]], base=0,
                       channel_multiplier=1)
        bcol4f = sb.tile([B, 1], f32)
        nc.vector.tensor_copy(bcol4f[:], bcol4[:])
        j32 = sb.tile([B, 128], f32)
        nc.vector.tensor_scalar(j32[:], col128[:], 1.0 / 32.0, None,
                                op0=Alu.mult)
        floor_f(j32[:], [B, 128], "fl4")
        L4 = sb.tile([B, 128], f32)
        nc.vector.tensor_tensor(L4[:], j32[:],
                                bcol4f[:].to_broadcast([B, 128]),
                                op=Alu.is_equal)

        # S4[p, j] = 1 iff j == 8*(p//32) + p%8
        col32_i = sb.tile([128, 32], i32)
        nc.gpsimd.iota(col32_i[:], pattern=[[1, 32]], base=0,
                       channel_multiplier=0)
        col32 = sb.tile([128, 32], f32)
        nc.vector.tensor_copy(col32[:], col32_i[:])
        jtgt = sb.tile([128, 1], f32)
        nc.vector.scalar_tensor_tensor(jtgt[:], in0=b_of_p[:], scalar=8.0,
                                       in1=s8_of_p[:], op0=Alu.mult,
                                       op1=Alu.add)
        S_CAT = sb.tile([128, 64], bf16)
        nc.vector.tensor_tensor(S_CAT[:, 0:32], col32[:],
                                jtgt[:].to_broadcast([128, 32]),
                                op=Alu.is_equal)

        # G2[p, c] = 1 iff c == (p%32)//8
        colC_i = sb.tile([128, C], i32)
        nc.gpsimd.iota(colC_i[:], pattern=[[1, C]], base=0,
                       channel_multiplier=0)
        colC = sb.tile([128, C], f32)
        nc.vector.tensor_copy(colC[:], colC_i[:])
        c_of_p = sb.tile([128, 1], f32)
        nc.vector.tensor_scalar(c_of_p[:], r32_of_p[:], 1.0 / 8.0, None,
                                op0=Alu.mult)
        floor_f(c_of_p[:], [128, 1], "fg2")
        G2 = sb.tile([128, C], f32)
        nc.vector.tensor_tensor(G2[:], colC[:],
                                c_of_p[:].to_broadcast([128, C]),
                                op=Alu.is_equal)
        # S0 = S4 masked to c==0 rows: S_CAT[:, 32:64]
        c0ind = sb.tile([128, 1], f32)
        nc.vector.tensor_scalar(c0ind[:], c_of_p[:], 0.0, None,
                                op0=Alu.is_equal)
        nc.vector.tensor_tensor(S_CAT[:, 32:64], S_CAT[:, 0:32],
                                c0ind[:].to_broadcast([128, 32]),
                                op=Alu.mult)

        ones8 = sb.tile([NCORES, 1], f32)
        nc.gpsimd.memset(ones8[:], 1.0)
        ones104 = sb.tile([1, KE], f32)
        nc.gpsimd.memset(ones104[:], 1.0)
        p1024 = sb.tile([128, 1], f32)
        nc.vector.tensor_scalar(p1024[:], iota_pf[:], 1024.0, None,
                                op0=Alu.mult)
        bsub = sb.tile([128, 1], f32)
        nc.vector.tensor_scalar(bsub[:], b_of_p[:], float(VL), None,
                                op0=Alu.mult)
        # bcol[k, b] = b*VL
        bcol_i = sb.tile([KE, B], i32)
        nc.gpsimd.iota(bcol_i[:], pattern=[[1, B]], base=0,
                       channel_multiplier=0)
        bcol = sb.tile([KE, B], f32)
        nc.vector.tensor_copy(bcol[:], bcol_i[:])
        nc.vector.tensor_scalar_mul(bcol[:], bcol[:], float(VL))
        # bcol2[k, b] = (b//2)*(2*VL) + b%2  (he-gather row base)
        bcol2 = sb.tile([KE, B], f32)
        nc.vector.tensor_copy(bcol2[:], bcol_i[:])
        nc.vector.tensor_scalar_mul(bcol2[:], bcol2[:], 0.5)
        floor_f(bcol2[:], [KE, B], "fbc2")
        halfb = sb.tile([KE, B], f32)
        nc.vector.tensor_copy(halfb[:], bcol_i[:])
        nc.vector.scalar_tensor_tensor(halfb[:], in0=bcol2[:], scalar=-2.0,
                                       in1=halfb[:], op0=Alu.mult,
                                       op1=Alu.add)
        nc.vector.tensor_scalar_mul(bcol2[:], bcol2[:], float(2 * VL))
        nc.vector.tensor_tensor(bcol2[:], bcol2[:], halfb[:], op=Alu.add)

        # preload ACT tables for the tail's Exp/Ln (avoids 1.3us
        # ACT_TABLE_LOAD on the critical path)
        warm = sb.tile([1, 8], f32)
        nc.gpsimd.memset(warm[:], 0.5)
        warm2 = sb.tile([1, 8], f32)
        nc.scalar.activation(warm2[:], warm[:], Act.Exp, scale=1.0)
        nc.scalar.activation(warm2[:], warm2[:], Act.Ln)

        # ============ candidate path (DVE) ============
        # Worig/Wx live inside proba_sb (c=0 / c=1 slots) to save SBUF
        Worig = proba_sb[:, 0, :]
        nc.vector.tensor_tensor(Worig, proba_sb[:, 0, :],
                                proba_sb[:, 1, :], op=Alu.mult)
        nc.vector.tensor_tensor(Worig, Worig, proba_sb[:, 2, :],
                                op=Alu.mult)
        nc.vector.tensor_tensor(Worig, Worig, proba_sb[:, 3, :],
                                op=Alu.mult)
        Wx = proba_sb[:, 1, :]
        nc.vector.tensor_copy(Wx, proba_sb[:, 0, :])
        V1 = sb.tile([128, 8 * R1], f32)
        for r in range(R1):
            sl = V1[:, r * 8:(r + 1) * 8]
            nc.vector.max(out=sl, in_=Wx)
            nc.vector.match_replace(out=Wx, in_to_replace=sl,
                                    in_values=Wx, imm_value=SENT)
        cand = sb.tile([B, 32 * 8 * R1], f32)
        nc.gpsimd.dma_start(cand[:], V1[:])
        L2V = sb.tile([B, NL], f32)
        for r in range(R2):
            sl = L2V[:, r * 8:(r + 1) * 8]
            nc.vector.max(out=sl, in_=cand[:])
            nc.vector.match_replace(out=cand[:], in_to_replace=sl,
                                    in_values=cand[:], imm_value=SENT)
        emit_tap("d_L2V", L2V[:], [B, NL])
        nc.gpsimd.dma_start(b1_in[:], L2V[:])
        # AllGather #1 — fired early; blocks gpsimd until done (~45us);
        # nothing else is queued on gpsimd until gcand/tail.
        nc.gpsimd.collective_compute(
            "AllGather", Alu.bypass,
            replica_groups=[list(range(NCORES))],
            ins=[b1_in[:].opt()], outs=[b1_out[:].opt()])

        # ============ mask path (fp8) ============
        maskT = sb.tile([128, 32, 128], fp8)
        for g in range(8):
            mt_ps = ps.tile([128, 512], bf16, tag="pstr", bufs=3,
                            name=f"mt{g}")
            for q in range(4):
                s = g * 4 + q
                nc.tensor.transpose(mt_ps[:, q * 128:(q + 1) * 128],
                                    mask8[:, s * 128:(s + 1) * 128],
                                    ident_bf[:])
            nc.scalar.activation(maskT[:, g * 4:(g + 1) * 4, :], mt_ps[:],
                                 Act.Copy)
        emit_tap("d_maskT", maskT[:, 0, :], [128, 128], fp8)

        cnt_part = sb.tile([128, 1], f32)
        nc.vector.reduce_sum(cnt_part[:], mask8[:], axis=Axis.X)
        cnt_ps = ps.tile([C, 1], f32, tag="psa")
        nc.tensor.matmul(cnt_ps[:], lhsT=G2[:], rhs=cnt_part[:],
                         start=True, stop=True)
        cnt_sb = sb.tile([C, 1], f32)
        nc.vector.tensor_copy(cnt_sb[:], cnt_ps[:])
        emit_tap("d_cnt", cnt_sb[:], [C, 1])

        # sel field: s[b,v] = 1 if (no class active) or (class0 active)
        # one matmul computes [msum | m0] stacked on 64 psum partitions
        sval = sb.tile([32, 4096], f32)
        for q in range(8):
            ms_ps = ps.tile([64, 512], f32, tag="psb", bufs=1,
                            name=f"ms{q}")
            nc.tensor.matmul(ms_ps[:], lhsT=S_CAT[:],
                             rhs=mask8[:, 512 * q:512 * (q + 1)],
                             start=True, stop=True)
            m0sb = sb2.tile([32, 512], f32, tag="m0sb")
            nc.vector.tensor_copy(m0sb[:], ms_ps[32:64, :])
            nc.vector.scalar_tensor_tensor(
                sval[:, 512 * q:512 * (q + 1)], in0=ms_ps[0:32, :],
                scalar=0.0, in1=m0sb[:], op0=Alu.is_equal,
                op1=Alu.max)
        nc.gpsimd.dma_start(
            s_dram[:].rearrange("(p v) a -> p (v a)", p=32), sval[:])

        # ============ emb streaming ============
        csA = ps.tile([8, 128], f32, tag="pscsA", bufs=1)
        csB = ps.tile([8, 128], f32, tag="pscsB", bufs=1)
        first_mm = {0: True, 1: True}
        n_mm = {0: 0, 1: 0}

        def stream_chunk(ci):
            p, h = ci % 2, ci // 2
            t = chunks.pop(ci)
            ebT = sbt.tile([128, 64, 128], fp8, tag="ebT", name=f"ebT{ci}")
            cs_psum = csA if p == 0 else csB
            for g in range(16):
                tp_ps = ps.tile([128, 512], f32, tag="pstr", bufs=3,
                                name=f"tp{ci}_{g}")
                for q in range(4):
                    vb = g * 4 + q
                    nc.tensor.transpose(tp_ps[:, q * 128:(q + 1) * 128],
                                        t[:, vb * 128:(vb + 1) * 128],
                                        ident[:])
                dst = ebT[:, g * 4:(g + 1) * 4, :]
                nc.scalar.activation(dst, tp_ps[:], Act.Copy)
                for q in range(4):
                    vb = g * 4 + q
                    v0 = h * CH + vb * 128
                    s8, s = v0 // 4096, (v0 % 4096) // 128
                    c0 = s8 + 64 * p
                    lhsT = maskT[:, s, c0:c0 + 57:8]  # 8 cols: pair's (b,c)
                    n_mm[p] += 1
                    nc.tensor.matmul(
                        cs_psum[:], lhsT=lhsT,
                        rhs=ebT[:, vb, :],
                        start=first_mm[p],
                        stop=(n_mm[p] == (VL // 128)),
                        skip_group_check=True)
                    first_mm[p] = False
            # eviction: one DMA, 8KB contiguous per partition both sides
            nc.sync.dma_start(
                embT_dram[p, :, h * 64:(h + 1) * 64, :, :],
                ebT[:].rearrange("vp t (hf f) -> vp t hf f", hf=2))
            if ci + 2 < NCHUNK:
                load_chunk(ci + 2)

        for ci in range(4):
            stream_chunk(ci)

        # ===== merge + index recovery (mid-stream; gcand via gpsimd) =====
        gcand = sb.tile([B, NCORES * NL], f32)
        nc.gpsimd.dma_start(
            gcand[:].rearrange("b (ci r) -> b ci r", ci=NCORES),
            b1_out[:].rearrange("ci b r -> b ci r"))
        emit_tap("d_gcand", gcand[:], [B, NCORES * NL])
        G = sb.tile([B, KE], f32)
        for r in range(R3):
            sl = G[:, r * 8:(r + 1) * 8]
            nc.vector.max(out=sl, in_=gcand[:])
            nc.vector.match_replace(out=gcand[:], in_to_replace=sl,
                                    in_values=gcand[:], imm_value=SENT)
        emit_tap("d_G", G[:], [B, KE])

        bG_ps = ps.tile([128, KE], f32, tag="psa")
        nc.tensor.matmul(bG_ps[:], lhsT=L4[:], rhs=G[:], start=True,
                         stop=True)
        bG = sb.tile([128, KE], f32)
        nc.vector.tensor_copy(bG[:], bG_ps[:])
        idxu = sb.tile([128, KE], u32)
        for r in range(R3):
            nc.vector.max_index(out=idxu[:, r * 8:(r + 1) * 8],
                                in_max=bG[:, r * 8:(r + 1) * 8],
                                in_values=Worig)
        idxf = sb.tile([128, KE], f32)
        nc.vector.tensor_copy(idxf[:], idxu[:])
        nc.vector.tensor_tensor(idxf[:], idxf[:],
                                p1024[:].to_broadcast([128, KE]), op=Alu.add)
        nc.vector.tensor_tensor(idxf[:], idxf[:],
                                bsub[:].to_broadcast([128, KE]),
                                op=Alu.subtract)
        tidx_ps = ps.tile([KE, 128], f32, tag="psa")
        nc.tensor.transpose(tidx_ps[:], idxf[:], ident[:])
        tidx = sb.tile([KE, 128], f32)
        nc.vector.tensor_copy(tidx[:], tidx_ps[:])
        lminT = sb.tile([KE, B], f32)
        for b in range(B):
            nc.vector.tensor_reduce(lminT[:, b:b + 1],
                                    tidx[:, 32 * b:32 * b + 32],
                                    axis=Axis.X, op=Alu.min)
        emit_tap("d_lminT", lminT[:], [KE, B])

        # own = (local voxel id found), gather rows:
        # sel row: b*VL + v (flat);  he row: b*VL + (v%128)*TDIV + v//128
        own_k = sb.tile([KE, B], f32)
        nc.vector.tensor_scalar(own_k[:], lminT[:], 1.0e6, None,
                                op0=Alu.is_le)
        locv = sb.tile([KE, B], f32)
        nc.vector.tensor_scalar_min(locv[:], lminT[:], float(VL))

        idxs_f = sb.tile([KE, B], f32)
        nc.vector.tensor_tensor(idxs_f[:], locv[:], bcol[:], op=Alu.add)
        notown = sb.tile([KE, B], f32)
        nc.vector.tensor_scalar(notown[:], own_k[:], 0.0, None,
                                op0=Alu.is_equal)
        nc.vector.tensor_scalar_mul(notown[:], notown[:], NOTFOUND_CLAMP)
        nc.vector.tensor_tensor(idxs_f[:], idxs_f[:], notown[:], op=Alu.add)
        nc.vector.tensor_scalar_min(idxs_f[:], idxs_f[:], NOTFOUND_CLAMP)
        idxs_all = sb.tile([KE, B], i32)
        nc.vector.tensor_copy(idxs_all[:], idxs_f[:])

        vdiv = sb.tile([KE, B], f32)
        nc.vector.tensor_scalar(vdiv[:], locv[:], 1.0 / 128.0, None,
                                op0=Alu.mult)
        floor_f(vdiv[:], [KE, B], "fvd")
        vmod = sb.tile([KE, B], f32)
        nc.vector.scalar_tensor_tensor(vmod[:], in0=vdiv[:], scalar=-128.0,
                                       in1=locv[:], op0=Alu.mult,
                                       op1=Alu.add)
        row_f = sb.tile([KE, B], f32)
        nc.vector.scalar_tensor_tensor(row_f[:], in0=vdiv[:], scalar=2.0,
                                       in1=bcol2[:], op0=Alu.mult,
                                       op1=Alu.add)
        nc.vector.scalar_tensor_tensor(row_f[:], in0=vmod[:],
                                       scalar=float(2 * TDIV), in1=row_f[:],
                                       op0=Alu.mult, op1=Alu.add)
        nc.vector.tensor_tensor(row_f[:], row_f[:], notown[:], op=Alu.add)
        nc.vector.tensor_scalar_min(row_f[:], row_f[:], NOTFOUND_CLAMP)
        idxe_all = sb.tile([KE, B], i32)
        nc.vector.tensor_copy(idxe_all[:], row_f[:])
        emit_tap("d_idxe", row_f[:], [KE, B])

        he_all = sb.tile([KE, B, F], fp8)
        nc.vector.memset(he_all[:], 0.0)
        sel_all = sb.tile([KE, B], f32)
        nc.vector.memset(sel_all[:], 0.0)

        for ci in range(4, NCHUNK):
            stream_chunk(ci)

        # ============ tail: gathers + AllGather#2 (gpsimd FIFO) ============
        embT_flat = embT_dram[:].rearrange("q p t hf f -> (q p t hf) f")
        for b in range(B):
            nc.gpsimd.indirect_dma_start(
                out=he_all[:, b, :], out_offset=None,
                in_=embT_flat,
                in_offset=IndirectOffsetOnAxis(ap=idxe_all[:, b:b + 1],
                                               axis=0),
                bounds_check=B * VL - 1, oob_is_err=False)
            nc.gpsimd.indirect_dma_start(
                out=sel_all[:, b:b + 1], out_offset=None,
                in_=s_dram[:],
                in_offset=IndirectOffsetOnAxis(ap=idxs_all[:, b:b + 1],
                                               axis=0),
                bounds_check=B * VL - 1, oob_is_err=False)
        emit_tap("d_he0", he_all[:, 0, :], [KE, F], fp8)
        emit_tap("d_sel0", sel_all[:, 0:1], [KE, 1])

        # class_sum extract: fold the 4 valid [4,64] quadrants via PE
        # (32-aligned partition rule forbids direct offset-partition adds)
        csA_sb = sb.tile([8, 128], bf16)
        nc.vector.tensor_copy(csA_sb[:], csA[:])
        csB_sb = sb.tile([8, 128], bf16)
        nc.vector.tensor_copy(csB_sb[:], csB[:])
        selb0 = ident_bf[0:8, 0:4]      # picks rows j=c
        selb1 = ident_bf[0:8, 4:8]      # picks rows j=4+c
        cls_ps = ps.tile([C, F], f32, tag="psa")
        nc.tensor.matmul(cls_ps[:], lhsT=selb0, rhs=csA_sb[:, 0:F],
                         start=True, stop=False, skip_group_check=True)
        nc.tensor.matmul(cls_ps[:], lhsT=selb1, rhs=csA_sb[:, F:2 * F],
                         start=False, stop=False, skip_group_check=True)
        nc.tensor.matmul(cls_ps[:], lhsT=selb0, rhs=csB_sb[:, 0:F],
                         start=False, stop=False, skip_group_check=True)
        nc.tensor.matmul(cls_ps[:], lhsT=selb1, rhs=csB_sb[:, F:2 * F],
                         start=False, stop=True, skip_group_check=True)
        pack2 = sb.tile([C, F + 1], f32)
        nc.vector.tensor_copy(pack2[:, 0:F], cls_ps[:])
        nc.vector.tensor_copy(pack2[:, F:F + 1], cnt_sb[:])
        emit_tap("d_cs", pack2[:, 0:F], [C, F])
        nc.scalar.dma_start(b2_in[:], pack2[:])
        nc.gpsimd.collective_compute(
            "AllGather", Alu.bypass,
            replica_groups=[list(range(NCORES))],
            ins=[b2_in[:].opt()], outs=[b2_out[:].opt()])

        # ============ EMA -> nar ============
        g2t = sb.tile([NCORES, C * (F + 1)], f32)
        nc.scalar.dma_start(g2t[:], b2_out[:])
        gsum_ps = ps.tile([1, C * (F + 1)], f32, tag="psa")
        nc.tensor.matmul(gsum_ps[:], lhsT=ones8[:], rhs=g2t[:],
                         start=True, stop=True)
        gsum = sb.tile([1, C * (F + 1)], f32)
        nc.vector.tensor_copy(gsum[:], gsum_ps[:])
        emit_tap("d_gsum", gsum[:], [1, C * (F + 1)])
        cs_g = gsum[:].rearrange("a (c f) -> a c f", c=C)[:, :, 0:F]
        cnt_g = gsum[:].rearrange("a (c f) -> a c f", c=C)[:, :, F:F + 1]

        cntc = sb.tile([1, C, 1], f32)
        nc.vector.tensor_scalar_max(cntc[:], cnt_g, 1.0)
        recip = sb.tile([1, C, 1], f32)
        nc.vector.reciprocal(recip[:], cntc[:])
        mean = sb.tile([1, C, F], f32)
        nc.vector.tensor_tensor(mean[:], cs_g,
                                recip[:].to_broadcast([1, C, F]), op=Alu.mult)
        avg_sb = sb.tile([1, C, F], f32)
        nc.scalar.dma_start(avg_sb[:], avg_d)
        cgt = sb.tile([1, C, 1], f32)
        nc.vector.tensor_scalar(cgt[:], cnt_g, 0.0, None, op0=Alu.is_gt)
        t_ema = sb.tile([1, C, F], f32)
        nc.vector.tensor_tensor(t_ema[:], mean[:], avg_sb[:],
                                op=Alu.subtract)
        nc.vector.scalar_tensor_tensor(t_ema[:], in0=t_ema[:],
                                       scalar=EMA_THETA,
                                       in1=cgt[:].to_broadcast([1, C, F]),
                                       op0=Alu.mult, op1=Alu.mult)
        avg_new = sb.tile([1, C, F], f32)
        nc.vector.tensor_tensor(avg_new[:], avg_sb[:], t_ema[:], op=Alu.add)

        narb_ps = ps.tile([KE, (C - 1) * F], f32, tag="psb", bufs=1)
        nc.tensor.matmul(narb_ps[:], lhsT=ones104[:], rhs=avg_new[:, 1:C, :],
                         start=True, stop=True)
        narb = sb.tile([KE, C - 1, F], f32)
        nc.vector.tensor_copy(narb[:], narb_ps[:])
        emit_tap("d_narb", narb[:], [KE, (C - 1) * F])

        # ============ loss epilogue ============
        hef = sb.tile([KE, B, F], f32)
        nc.vector.tensor_copy(hef[:], he_all[:])
        prod = sb.tile([KE, B, C - 1, F], f32)
        nc.vector.tensor_tensor(
            prod[:],
            hef[:].rearrange("k b (o f) -> k b o f", o=1).to_broadcast(
                [KE, B, C - 1, F]),
            narb[:].rearrange("k (o c) f -> k o c f", o=1).to_broadcast(
                [KE, B, C - 1, F]),
            op=Alu.mult)
        ex = sb.tile([KE, B, C - 1, F], f32)
        nc.scalar.activation(ex[:], prod[:], Act.Exp, scale=1.0 / TAU)
        accl = sb.tile([KE, B, F], f32)
        nc.vector.tensor_tensor(accl[:], ex[:, :, 0, :], ex[:, :, 1, :],
                                op=Alu.add)
        nc.vector.tensor_tensor(accl[:], accl[:], ex[:, :, 2, :], op=Alu.add)
        lnv = sb.tile([KE, B, F], f32)
        nc.scalar.activation(lnv[:], accl[:], Act.Ln)
        lsum = sb.tile([KE, B], f32)
        nc.vector.reduce_sum(lsum[:], lnv[:], axis=Axis.X)
        nc.vector.tensor_scalar_mul(lsum[:], lsum[:], -1.0)
        nc.vector.tensor_tensor(lsum[:], lsum[:], sel_all[:], op=Alu.mult)
        losscols = sb.tile([KE, B], f32)
        nc.vector.tensor_tensor(losscols[:], lsum[:], own_k[:], op=Alu.mult)

        final_ps = ps.tile([1, B * KE], f32, tag="psa")
        for b in range(B):
            nc.tensor.matmul(final_ps[:, b * KE:(b + 1) * KE],
                             lhsT=losscols[:, b:b + 1], rhs=ident[:KE, :KE],
                             start=True, stop=True)
        final = sb.tile([1, B * KE], f32)
        nc.vector.tensor_copy(final[:], final_ps[:])
        nc.sync.dma_start(out_d, final[:])


_NC_CACHE = {}


def _get_graph():
    if "nc" not in _NC_CACHE:
        _NC_CACHE["nc"] = build_graph()
    return _NC_CACHE["nc"]


def kernel(proba, y, embeddings, avg_repr):
    from concourse.bass_utils import run_bass_kernel_spmd

    proba = np.asarray(proba, dtype=np.float32)
    y = np.asarray(y, dtype=np.int32)
    embeddings = np.asarray(embeddings, dtype=np.float32)
    avg_repr = np.asarray(avg_repr, dtype=np.float32)

    nc = _get_graph()
    in_maps = []
    for ci in range(NCORES):
        sl = slice(ci * VL, (ci + 1) * VL)
        in_maps.append({
            "proba": np.ascontiguousarray(proba[:, :, sl]),
            "y": np.ascontiguousarray(y[:, :, sl]),
            "embeddings": np.ascontiguousarray(embeddings[:, :, sl]),
            "avg_repr": avg_repr,
        })
    res = run_bass_kernel_spmd(nc, in_maps, core_ids=list(range(NCORES)))
    parts = [res.results[ci]["out"] for ci in range(NCORES)]
    total = np.sum(parts, axis=0).astype(np.float32)
    return total[:, :K]
